# revision 1
# baseline (speedup 1.0000x reference)
"""3x3 median filter (reflect padding) on Trainium2, 8-core data parallel.

Input  x: (4, 3, 1024, 1024) float32
Output  : (4, 3, 1024, 1024) float32  (Kornia MedianBlur semantics)

Strategy:
  - Host: reflect-pad H and W by 1 -> (12, 1026, 1026); shard H across 8
    cores with 1-row halo: core k gets padded rows [128k, 128k+130).
  - Device (per core): for each of the 12 images, load three partition-
    aligned tiles T0/T1/T2 = rows p, p+1, p+2 (so vertical neighbors are
    partition-aligned, no cross-partition ops needed), then the classic
    sorted-column median network (18 min/max ops/pixel amortized):
      column sort (vertical):  lo/mid/hi of each 3-row column
      horizontal merge:        med3(max3(lo), med3(mid), min3(hi))
    All min/max in fp32 -> result is bit-exact vs the reference.
"""

import sys

sys.path.insert(0, "/opt/trn_rl_repo")

import numpy as np

B, C, H, W = 4, 3, 1024, 1024
NIMG = B * C            # 12
NCORES = 8
ROWS_PER_CORE = H // NCORES   # 128
WP = W + 2              # 1026 padded width
HP_CORE = ROWS_PER_CORE + 2   # 130 padded rows per core

_PROGRAM = None
LAST_RESULT = None


def _build_program():
    import concourse.bacc as bacc
    import concourse.tile as tile
    import concourse.mybir as mybir
    from contextlib import ExitStack

    f32 = mybir.dt.float32
    mn = mybir.AluOpType.min
    mx = mybir.AluOpType.max

    nc = bacc.Bacc("TRN2", target_bir_lowering=False, debug=False,
                   num_devices=NCORES)
    x = nc.dram_tensor("x", [NIMG, HP_CORE, WP], f32, kind="ExternalInput").ap()
    y = nc.dram_tensor("y", [NIMG, ROWS_PER_CORE, W], f32,
                       kind="ExternalOutput").ap()

    P = ROWS_PER_CORE  # 128 partitions

    with tile.TileContext(nc) as tc, ExitStack() as ctx:
        pool = ctx.enter_context(tc.tile_pool(name="p", bufs=2))

        def tt(dst, a, b, op):
            nc.any.tensor_tensor(dst, a, b, op=op)

        for i in range(NIMG):
            T0 = pool.tile([P, WP], f32, tag="T0")
            T1 = pool.tile([P, WP], f32, tag="T1")
            T2 = pool.tile([P, WP], f32, tag="T2")
            nc.sync.dma_start(T0[:], x[i, 0:P, :])
            nc.sync.dma_start(T1[:], x[i, 1:P + 1, :])
            nc.sync.dma_start(T2[:], x[i, 2:P + 2, :])

            # vertical column sort: lo/mid/hi over rows (p, p+1, p+2)
            m = pool.tile([P, WP], f32, tag="m")
            M = pool.tile([P, WP], f32, tag="M")
            tt(m[:], T0[:], T1[:], mn)
            tt(M[:], T0[:], T1[:], mx)
            lo = pool.tile([P, WP], f32, tag="lo")
            mm = pool.tile([P, WP], f32, tag="mm")
            hi = pool.tile([P, WP], f32, tag="hi")
            mid = pool.tile([P, WP], f32, tag="mid")
            tt(lo[:], m[:], T2[:], mn)
            tt(mm[:], M[:], T2[:], mn)
            tt(hi[:], M[:], T2[:], mx)
            tt(mid[:], m[:], mm[:], mx)

            # horizontal: A = max3(lo), C = min3(hi), B = med3(mid)
            pa = pool.tile([P, W + 1], f32, tag="pa")
            A = pool.tile([P, W], f32, tag="A")
            tt(pa[:], lo[:, 0:W + 1], lo[:, 1:W + 2], mx)
            tt(A[:], pa[:, 0:W], lo[:, 2:W + 2], mx)

            pc = pool.tile([P, W + 1], f32, tag="pc")
            Cm = pool.tile([P, W], f32, tag="Cm")
            tt(pc[:], hi[:, 0:W + 1], hi[:, 1:W + 2], mn)
            tt(Cm[:], pc[:, 0:W], hi[:, 2:W + 2], mn)

            pm = pool.tile([P, W + 1], f32, tag="pm")
            pM = pool.tile([P, W + 1], f32, tag="pM")
            tt(pm[:], mid[:, 0:W + 1], mid[:, 1:W + 2], mn)
            tt(pM[:], mid[:, 0:W + 1], mid[:, 1:W + 2], mx)
            t2 = pool.tile([P, W], f32, tag="t2")
            Bm = pool.tile([P, W], f32, tag="Bm")
            tt(t2[:], pM[:, 0:W], mid[:, 2:W + 2], mn)
            tt(Bm[:], pm[:, 0:W], t2[:], mx)

            # final med3(A, B, C)
            m1 = pool.tile([P, W], f32, tag="m1")
            M1 = pool.tile([P, W], f32, tag="M1")
            tt(m1[:], A[:], Bm[:], mn)
            tt(M1[:], A[:], Bm[:], mx)
            t3 = pool.tile([P, W], f32, tag="t3")
            out = pool.tile([P, W], f32, tag="out")
            tt(t3[:], M1[:], Cm[:], mn)
            tt(out[:], m1[:], t3[:], mx)

            nc.sync.dma_start(y[i], out[:])

    nc.compile()
    return nc


def _get_program():
    global _PROGRAM
    if _PROGRAM is None:
        _PROGRAM = _build_program()
    return _PROGRAM


def kernel(x):
    global LAST_RESULT
    from concourse.bass_utils import run_bass_kernel_spmd
    import os

    x = np.asarray(x, dtype=np.float32)
    xp = np.pad(x.reshape(NIMG, H, W), ((0, 0), (1, 1), (1, 1)),
                mode="reflect")
    in_maps = [
        {"x": np.ascontiguousarray(
            xp[:, ROWS_PER_CORE * k: ROWS_PER_CORE * k + HP_CORE, :])}
        for k in range(NCORES)
    ]
    nc = _get_program()
    trace = bool(int(os.environ.get("MEDIAN_TRACE", "0")))
    res = run_bass_kernel_spmd(nc, in_maps, list(range(NCORES)), trace=trace)
    LAST_RESULT = res
    out = np.concatenate([res.results[k]["y"] for k in range(NCORES)], axis=1)
    return out.reshape(B, C, H, W)



# revision 5
# speedup vs baseline: 1.8467x; 1.8467x over previous
"""3x3 median filter (reflect padding) on Trainium2, 8-core data parallel.

Input  x: (4, 3, 1024, 1024) float32
Output  : (4, 3, 1024, 1024) float32  (Kornia MedianBlur semantics)

Strategy (v2):
  - Host: cast to fp16 (tolerance 2e-2 >> fp16 eps), reflect-pad H/W by 1
    -> (12, 1026, 1026); shard H across 8 cores: core k gets padded rows
    [128k, 128k+130).
  - Device (per core): images processed in groups of G along the free dim.
    Classic sorted-column median network (18 min/max ops per pixel):
      vertical sort3 of rows (6 ops), horizontal merge (12 ops).
    Engine split: 13 ops on DVE (all operands 4B-aligned unit-stride fp16
    -> 2x perf mode) + 5 ops on GpSimd (including the four odd-offset
    pair ops, which would drop DVE to 1x mode).
  - fp16 halves HBM traffic and doubles DVE throughput vs the fp32
    baseline; min/max network is order-exact, so the only error is the
    fp16 input rounding (~1e-3 relative).
"""

import sys

sys.path.insert(0, "/opt/trn_rl_repo")

import numpy as np

B, C, H, W = 4, 3, 1024, 1024
NIMG = B * C            # 12
NCORES = 8
ROWS_PER_CORE = H // NCORES   # 128
WP = W + 2              # 1026 padded width
HP_CORE = ROWS_PER_CORE + 2   # 130 padded rows per core
G = 2                   # images per op group
NGROUPS = NIMG // G

_PROGRAM = None
LAST_RESULT = None


def _build_program():
    import concourse.bacc as bacc
    import concourse.tile as tile
    import concourse.mybir as mybir
    from contextlib import ExitStack

    f16 = mybir.dt.float16
    mn = mybir.AluOpType.min
    mx = mybir.AluOpType.max

    nc = bacc.Bacc("TRN2", target_bir_lowering=False, debug=False,
                   num_devices=NCORES)
    x = nc.dram_tensor("x", [NIMG, HP_CORE, WP], f16, kind="ExternalInput").ap()
    y = nc.dram_tensor("y", [NIMG, ROWS_PER_CORE, W], f16,
                       kind="ExternalOutput").ap()

    P = ROWS_PER_CORE  # 128 partitions

    with tile.TileContext(nc) as tc, ExitStack() as ctx:
        pool = ctx.enter_context(tc.tile_pool(name="p", bufs=2))

        # Per-group stage emitters. Tiles are [P, G, width]; ops run over
        # the full group (free size G*width) to amortize instruction
        # overhead. All DVE ops keep 4B-aligned unit-stride operands so
        # the 2x fp16 perf mode engages; odd-offset ops go to GpSimd.
        state = {}

        def load(g):
            T0 = pool.tile([P, G, WP], f16, tag="T0")
            T1 = pool.tile([P, G, WP], f16, tag="T1")
            T2 = pool.tile([P, G, WP], f16, tag="T2")
            for j in range(G):
                i = g * G + j
                nc.sync.dma_start(T0[:, j], x[i, 0:P, :])
                nc.sync.dma_start(T1[:, j], x[i, 1:P + 1, :])
                nc.sync.dma_start(T2[:, j], x[i, 2:P + 2, :])
            state[g] = {"T0": T0, "T1": T1, "T2": T2}

        def vertical(g):
            s = state[g]
            T0, T1, T2 = s["T0"], s["T1"], s["T2"]
            m = pool.tile([P, G, WP], f16, tag="m")
            M = pool.tile([P, G, WP], f16, tag="M")
            nc.vector.tensor_tensor(m[:], T0[:], T1[:], op=mn)
            nc.vector.tensor_tensor(M[:], T0[:], T1[:], op=mx)
            lo = pool.tile([P, G, WP], f16, tag="lo")
            mm = pool.tile([P, G, WP], f16, tag="mm")
            hi = pool.tile([P, G, WP], f16, tag="hi")
            mid = pool.tile([P, G, WP], f16, tag="mid")
            nc.vector.tensor_tensor(lo[:], m[:], T2[:], op=mn)
            nc.vector.tensor_tensor(mm[:], M[:], T2[:], op=mn)
            nc.vector.tensor_tensor(hi[:], M[:], T2[:], op=mx)
            nc.vector.tensor_tensor(mid[:], m[:], mm[:], op=mx)
            s["lo"], s["mid"], s["hi"] = lo, mid, hi

        def pool_pairs(g):
            # The four odd-offset pair ops -> GpSimd (alignment-agnostic).
            s = state[g]
            lo, mid, hi = s["lo"], s["mid"], s["hi"]
            # Width WP (even) rather than W+1 so row strides stay 4B-aligned
            # for the DVE consumers; only [0:W+1] is valid data.
            pa = pool.tile([P, G, WP], f16, tag="pa")
            pc = pool.tile([P, G, WP], f16, tag="pc")
            pm = pool.tile([P, G, WP], f16, tag="pm")
            pM = pool.tile([P, G, WP], f16, tag="pM")
            nc.vector.tensor_tensor(pa[:, :, 0:W + 1], lo[:, :, 0:W + 1], lo[:, :, 1:W + 2], op=mx)
            nc.vector.tensor_tensor(pc[:, :, 0:W + 1], hi[:, :, 0:W + 1], hi[:, :, 1:W + 2], op=mn)
            nc.vector.tensor_tensor(pm[:, :, 0:W + 1], mid[:, :, 0:W + 1], mid[:, :, 1:W + 2], op=mn)
            nc.vector.tensor_tensor(pM[:, :, 0:W + 1], mid[:, :, 0:W + 1], mid[:, :, 1:W + 2], op=mx)
            s["pa"], s["pc"], s["pm"], s["pM"] = pa, pc, pm, pM

        def horizontal(g):
            s = state[g]
            lo, mid, hi = s["lo"], s["mid"], s["hi"]
            pa, pc, pm, pM = s["pa"], s["pc"], s["pm"], s["pM"]
            A = pool.tile([P, G, W], f16, tag="A")
            Cm = pool.tile([P, G, W], f16, tag="Cm")
            t2 = pool.tile([P, G, W], f16, tag="t2")
            Bm = pool.tile([P, G, W], f16, tag="Bm")
            nc.vector.tensor_tensor(A[:], pa[:, :, 0:W], lo[:, :, 2:W + 2], op=mx)
            nc.vector.tensor_tensor(Cm[:], pc[:, :, 0:W], hi[:, :, 2:W + 2], op=mn)
            nc.vector.tensor_tensor(t2[:], pM[:, :, 0:W], mid[:, :, 2:W + 2], op=mn)
            nc.vector.tensor_tensor(Bm[:], pm[:, :, 0:W], t2[:], op=mx)
            m1 = pool.tile([P, G, W], f16, tag="m1")
            M1 = pool.tile([P, G, W], f16, tag="M1")
            t3 = pool.tile([P, G, W], f16, tag="t3")
            nc.vector.tensor_tensor(m1[:], A[:], Bm[:], op=mn)
            nc.vector.tensor_tensor(M1[:], A[:], Bm[:], op=mx)
            nc.vector.tensor_tensor(t3[:], M1[:], Cm[:], op=mn)
            s["m1"], s["t3"] = m1, t3

        def final(g):
            s = state[g]
            out = pool.tile([P, G, W], f16, tag="out")
            nc.vector.tensor_tensor(out[:], s["m1"][:], s["t3"][:], op=mx)
            for j in range(G):
                nc.sync.dma_start(y[g * G + j], out[:, j])
            del state[g]

        # Software-pipelined emission: while GpSimd chews group g's pair
        # ops, DVE starts group g+1's vertical stage.
        load(0)
        vertical(0)
        pool_pairs(0)
        for g in range(1, NGROUPS):
            load(g)
            vertical(g)
            pool_pairs(g)
            horizontal(g - 1)
            final(g - 1)
        horizontal(NGROUPS - 1)
        final(NGROUPS - 1)

    nc.compile()
    return nc


def _get_program():
    global _PROGRAM
    if _PROGRAM is None:
        _PROGRAM = _build_program()
    return _PROGRAM


def kernel(x):
    global LAST_RESULT
    from concourse.bass_utils import run_bass_kernel_spmd
    import os

    x = np.asarray(x, dtype=np.float32)
    xp = np.pad(x.reshape(NIMG, H, W), ((0, 0), (1, 1), (1, 1)),
                mode="reflect").astype(np.float16)
    in_maps = [
        {"x": np.ascontiguousarray(
            xp[:, ROWS_PER_CORE * k: ROWS_PER_CORE * k + HP_CORE, :])}
        for k in range(NCORES)
    ]
    nc = _get_program()
    trace = bool(int(os.environ.get("MEDIAN_TRACE", "0")))
    res = run_bass_kernel_spmd(nc, in_maps, list(range(NCORES)), trace=trace)
    LAST_RESULT = res
    out = np.concatenate([res.results[k]["y"] for k in range(NCORES)], axis=1)
    return out.reshape(B, C, H, W).astype(np.float32)
